# revision 15
# baseline (speedup 1.0000x reference)
"""Trainium2 Bass kernel for nn_CasparLayer (embedding -> GRU(reset_after) -> dense).

Problem shapes: B=128, T=256, VOCAB=41, EMB=512, HID=1024.

Strategy (per NeuronCore, SPMD x8, 4-way data parallel x2 replicas):
  - Embedding + input projection fused into per-step one-hot matmuls against
    precomputed gcat = emb_table @ gru_kernel + biases (one-hot rows sum to 1).
  - Keras masking folded into the z-gate: z' = sigmoid(zpre + (1-m)*30).
  - Software-pipelined recurrence: the one-hot / bias matmuls of step t+1 and
    the (2-step batched, M=64) dense head execute on the PE during step t's
    sigmoid/blend tail, so the PE never idles long enough for HAM to
    re-throttle the clock.
  - h_new -> hT (stationary operand for the next step's matmuls) is done with
    a single DVE StreamTranspose (32x32 block transpose) per 128-column half.
    This works because the hidden contraction chunks are defined with a
    custom permutation: chunk c = f//32, partition p' = 32q + f%32 for hidden
    h = 256q + f; the weight rows are permuted on the host to match, making
    the in-place 32x32 block transpose of F-layout h_new produce hT exactly.
  - The z sigmoid + blend is split into two 128-column halves so the first
    transpose (and the next step's k=0..3 gate matmuls) start while the
    second half's blend still runs.

The harness contract: kernel(**inputs) takes full unsharded numpy inputs and
returns the full [128, 256, 41] float32 logits.
"""

import contextlib
import ctypes
import os
import sys
import types

sys.path.insert(0, "/opt/trn_rl_repo")

import numpy as np
import ml_dtypes

import bass_rust
import concourse.bass as bass
import concourse.tile as tile
from concourse import mybir

B = 128
T = 256
VOCAB = 41
EMB = 512
HID = 1024
N_CORES = 8
BQ = 32  # batch quarter per core (4-way data parallel, x2 replicas)
Q = 4    # PE column groups = hidden quarters
KC = 8   # hidden contraction chunks (custom permuted)
OH_WIN = 32  # one-hot SBUF prefetch window (steps)

F32 = mybir.dt.float32
BF16 = mybir.dt.bfloat16
AF = mybir.ActivationFunctionType


# ---------------------------------------------------------------------------
# Workaround: this walrus build accepts at most ONE sync wait per instruction;
# Tile attaches several. Hoist extras onto single-wait NOPs inserted before.
# ---------------------------------------------------------------------------
def _split_multiwaits(nc, max_waits: int = 1) -> int:
    n_split = 0
    for fn in nc.m.functions:
        for blk in fn.blocks:
            insts = blk.instructions
            i = 0
            while i < len(insts):
                ins = insts[i]
                si = ins.sync_info
                if si is not None and len(si.on_wait) > max_waits:
                    waits = list(si.on_wait)
                    keep = waits[-max_waits:]
                    hoist = waits[:-max_waits]
                    ins.sync_info = bass_rust.SyncInfo(
                        on_wait=keep, on_update=list(si.on_update)
                    )
                    for w in hoist:
                        nop = mybir.InstNoOp(
                            name=nc.get_next_instruction_name(),
                            sync_info=bass_rust.SyncInfo(on_wait=[w], on_update=[]),
                            bass_nofuse=True,
                            engine=ins.engine,
                            text_hint="wait_split",
                        )
                        nc.register_instruction(nop)
                        blk.instructions.insert(i, nop)
                        i += 1
                        n_split += 1
                i += 1
    return n_split


# ---------------------------------------------------------------------------
# Optional NTFF profiling under axon (the container's antenv stub lacks the
# hook registration module). Enabled via BASS_GRU_TRACE=1.
# ---------------------------------------------------------------------------
def _register_axon_profile_hook():
    so_path = "/opt/axon/libaxon_pjrt.so"
    if "antenv.axon_hooks" in sys.modules:
        return
    mod = types.ModuleType("antenv.axon_hooks")
    state = {"hook": None}
    mod.set_axon_ntff_profile_hook = lambda h: state.__setitem__("hook", h)
    mod.get_axon_ntff_profile_hook = lambda: state["hook"]
    sys.modules["antenv.axon_hooks"] = mod

    try:
        lib = ctypes.CDLL(so_path)
    except OSError:
        return
    if not hasattr(lib, "axon_start_nrt_profile"):
        return
    lib.axon_start_nrt_profile.argtypes = [
        ctypes.POINTER(ctypes.c_int64),
        ctypes.c_size_t,
    ]
    lib.axon_start_nrt_profile.restype = ctypes.c_int64
    lib.axon_stop_nrt_profile.argtypes = [ctypes.c_char_p]
    lib.axon_stop_nrt_profile.restype = ctypes.c_int64

    @contextlib.contextmanager
    def _hook_cm(output_dir, device_ids):
        import jax

        jax.devices()
        if device_ids:
            ids = (ctypes.c_int64 * len(device_ids))(*device_ids)
            rc = lib.axon_start_nrt_profile(ids, len(device_ids))
        else:
            rc = lib.axon_start_nrt_profile(None, 0)
        if rc != 0:
            raise RuntimeError(f"axon_start_nrt_profile rc={rc}")
        try:
            yield
        finally:
            n = lib.axon_stop_nrt_profile(str(output_dir).encode())
            print(f"ntff profile: {n} file(s) -> {output_dir}", file=sys.stderr)

    state["hook"] = _hook_cm

    import concourse.bass_utils as bu

    bu.upload_artifacts = lambda tmpdir: ""


# ---------------------------------------------------------------------------
# Kernel builder
# ---------------------------------------------------------------------------
def build_kernel(n_steps: int = T):
    assert n_steps % 2 == 0
    nc = bass.Bass()

    wzr_d = nc.declare_dram_parameter("wzr", [128, KC, 2048], BF16, isOutput=False)
    wh_d = nc.declare_dram_parameter("wh", [128, KC, 1024], BF16, isOutput=False)
    gzr_d = nc.declare_dram_parameter("gzr", [VOCAB, 2048], BF16, isOutput=False)
    gx_d = nc.declare_dram_parameter("gx", [VOCAB, 1024], BF16, isOutput=False)
    b1h_d = nc.declare_dram_parameter("b1h", [1, HID], BF16, isOutput=False)
    oh_d = nc.declare_dram_parameter("onehot", [n_steps, VOCAB, BQ], BF16, isOutput=False)
    zb_d = nc.declare_dram_parameter("zbias", [128, n_steps], F32, isOutput=False)
    dw_d = nc.declare_dram_parameter("dw", [128, KC, VOCAB], BF16, isOutput=False)
    db_d = nc.declare_dram_parameter("db", [1, VOCAB], BF16, isOutput=False)
    out_d = nc.declare_dram_parameter("logits", [n_steps, BQ, VOCAB], F32, isOutput=True)

    HQ = 256  # hidden columns per col-group section

    with tile.TileContext(nc) as tc:
        with contextlib.ExitStack() as ctx:
            singles = ctx.enter_context(tc.tile_pool(name="singles", bufs=1))
            state = ctx.enter_context(tc.tile_pool(name="state", bufs=1))
            temps = ctx.enter_context(tc.tile_pool(name="temps", bufs=2))
            ps_zr = ctx.enter_context(tc.tile_pool(name="ps_zr", bufs=3, space="PSUM"))
            ps_rx = ctx.enter_context(tc.tile_pool(name="ps_rx", bufs=3, space="PSUM"))
            ps_dn = ctx.enter_context(tc.tile_pool(name="ps_dn", bufs=2, space="PSUM"))

            # --- weights / constants resident in SBUF ---
            # (chunked DMAs so the first gate matmuls start before the full
            # weight load completes)
            wzr = singles.tile([128, KC, 2048], BF16)
            wh = singles.tile([128, KC, 1024], BF16)
            for c in range(KC):
                nc.sync.dma_start(out=wzr[:, c], in_=wzr_d[:, c])
                nc.sync.dma_start(out=wh[:, c], in_=wh_d[:, c])
            gzr = singles.tile([VOCAB, 2048], BF16)
            nc.sync.dma_start(out=gzr, in_=gzr_d[:])
            gx = singles.tile([VOCAB, 1024], BF16)
            nc.sync.dma_start(out=gx, in_=gx_d[:])
            b1h = singles.tile([1, HID], BF16)
            nc.sync.dma_start(out=b1h, in_=b1h_d[:])
            dw = singles.tile([128, KC, VOCAB], BF16)
            nc.sync.dma_start(out=dw, in_=dw_d[:])
            db = singles.tile([1, VOCAB], BF16)
            nc.sync.dma_start(out=db, in_=db_d[:])
            zb = singles.tile([128, n_steps], F32)
            nc.sync.dma_start(out=zb, in_=zb_d[:])
            ones32 = singles.tile([1, BQ], BF16)
            nc.vector.memset(ones32, 1.0)
            ones64 = singles.tile([1, 64], BF16)
            nc.vector.memset(ones64, 1.0)

            win = min(OH_WIN, n_steps)
            pd = max(1, win // 2)
            ohw = singles.tile([VOCAB, win, BQ], BF16)
            for t in range(min(pd, n_steps)):
                nc.sync.dma_start(out=ohw[:, t % win, :], in_=oh_d[t])

            # --- GRU state ---
            # h (F-layout [32q+b, f]): ping-pong by step parity.
            h_st = [state.tile([128, HQ], BF16, name=f"h{i}") for i in range(2)]
            # hT: both parity slots side by side so the dense head can read
            # two steps as one M=64 stationary operand. Slot s at [:, :, 32s:].
            hT = state.tile([128, KC, 64], BF16, name="hT")
            nc.vector.memset(h_st[1], 0.0)
            nc.vector.memset(hT, 0.0)

            def oh_group(out_ap, gtab, width, t1, start, stop):
                oh_ap = ohw[:, t1 % win, :]
                for j in range(Q):
                    nc.tensor.matmul(
                        out_ap[32 * j : 32 * (j + 1), :],
                        oh_ap,
                        gtab[:, j * width : (j + 1) * width],
                        start=start,
                        stop=stop,
                        tile_position=(0, 32 * j),
                    )

            def ones_group(out_ap):
                for j in range(Q):
                    nc.tensor.matmul(
                        out_ap[32 * j : 32 * (j + 1), :],
                        ones32,
                        b1h[:, j * HQ : (j + 1) * HQ],
                        start=True,
                        stop=False,
                        tile_position=(0, 32 * j),
                    )

            def gate_mms(out_ap, w, colw, coloff, slot, cs):
                # out_ap[32j:32j+32, :] += hT_slot.T @ w[:, c, colw*j + coloff ...]
                for c in cs:
                    for j in range(Q):
                        nc.tensor.matmul(
                            out_ap[32 * j : 32 * (j + 1), :],
                            hT[:, c, 32 * slot : 32 * slot + 32],
                            w[:, c, colw * j + coloff : colw * j + coloff + 256],
                            start=False,
                            stop=(c == KC - 1),
                            tile_position=(0, 32 * j),
                        )

            def alloc_ps(t1):
                # rx packs rh (cols 0:256) and xh (256:512) into one PSUM bank.
                zr_ps = ps_zr.tile([128, 2 * HQ], F32, tag="zr", name=f"zr{t1}")
                rx_ps = ps_rx.tile([128, 2 * HQ], F32, tag="rx", name=f"rx{t1}")
                return zr_ps, rx_ps

            def issue_A(t1, zr_ps, rx_ps):
                # one-hot / bias contributions for step t1 (no h dependency)
                oh_group(zr_ps, gzr, 512, t1, start=True, stop=False)
                oh_group(rx_ps[:, HQ:], gx, 256, t1, start=True, stop=True)
                ones_group(rx_ps[:, :HQ])

            def issue_C(t1, zr_ps, rx_ps, slot):
                # h-contraction matmuls + r/tanh chain for step t1
                s1 = t1 % 2
                gate_mms(zr_ps[:, HQ:], wzr, 512, 256, slot, range(KC))
                r_t = temps.tile([128, HQ], BF16, tag="r")
                nc.scalar.activation(r_t, zr_ps[:, HQ:], AF.Sigmoid)
                gate_mms(rx_ps[:, :HQ], wh, 256, 0, slot, range(KC))
                arg = temps.tile([128, HQ], BF16, tag="arg")
                nc.vector.tensor_mul(arg, r_t, rx_ps[:, :HQ])
                nc.vector.tensor_add(arg, arg, rx_ps[:, HQ:])
                hh = temps.tile([128, HQ], BF16, tag="hh")
                nc.scalar.activation(hh, arg, AF.Tanh)
                gate_mms(zr_ps[:, :HQ], wzr, 512, 0, slot, range(KC))
                d_t = temps.tile([128, HQ], BF16, tag="d")
                nc.vector.tensor_sub(d_t, h_st[1 - s1], hh)
                return hh, d_t

            def issue_B(t, zr_ps, hh, d_t):
                # step t's tail: z sigmoid + blend + transpose in halves
                s = t % 2
                for half in range(2):
                    lo = half * 128
                    z_h = temps.tile([128, 128], BF16, tag=f"z{half}")
                    nc.scalar.activation(
                        z_h, zr_ps[:, lo : lo + 128], AF.Sigmoid,
                        bias=zb[:, t : t + 1],
                    )
                    e_h = temps.tile([128, 128], BF16, tag=f"e{half}")
                    nc.vector.tensor_mul(e_h, z_h, d_t[:, lo : lo + 128])
                    nc.vector.tensor_add(h_st[s][:, lo : lo + 128],
                                         hh[:, lo : lo + 128], e_h)
                    nc.vector.transpose(
                        hT[:, 4 * half : 4 * half + 4, 32 * s : 32 * s + 32],
                        h_st[s][:, lo : lo + 128].rearrange("p (a b) -> p a b", a=4),
                    )

            def issue_D(t1):
                # dense head for step t1 (fill work for the current boundary)
                slot = t1 % 2
                dps = ps_dn.tile([BQ, VOCAB], F32, tag="dn", name=f"dn{t1}")
                for c in range(KC):
                    nc.tensor.matmul(
                        dps, hT[:, c, 32 * slot : 32 * slot + 32], dw[:, c, :],
                        start=(c == 0), stop=False,
                    )
                nc.tensor.matmul(dps, ones32, db, start=False, stop=True)
                lg = temps.tile([BQ, VOCAB], F32, tag="lg")
                nc.vector.tensor_copy(lg, dps)
                nc.sync.dma_start(out=out_d[t1], in_=lg)

            def issue_warm(ps, n, width):
                # Dummy K=1 matmuls streaming `width` cols into a corner of a
                # recycled PSUM tile (overwritten later by that step's real
                # start=True matmuls): keeps the PE's HAM activity monitor fed
                # across the boundary stall so the clock stays at 2.4 GHz.
                for _ in range(n):
                    nc.tensor.matmul(
                        ps[0:1, 0:width], ones32[:, 0:1], b1h[:, 0:width],
                        start=True, stop=True,
                    )

            # --- prologue: step 0 (hT slot 1 and h slot 1 are zeros) ---
            tiles = {t1: alloc_ps(t1) for t1 in range(min(3, n_steps))}
            issue_A(0, *tiles[0])
            if n_steps > 1:
                issue_A(1, *tiles[1])
            cur = issue_C(0, *tiles[0], slot=1)

            for t in range(n_steps):
                s = t % 2

                # B: step t's sigmoid/blend/transpose chain
                issue_B(t, tiles[t][0], *cur)

                # C: step t+1's gate matmuls (consume hT slot s as it appears)
                if t + 1 < n_steps:
                    cur = issue_C(t + 1, *tiles[t + 1], slot=s)

                # D: dense head for step t-1 (boundary fill)
                if t >= 1:
                    issue_D(t - 1)

                # A: one-hot/bias matmuls for step t+2, bracketed by keep-warm
                # dummies into recycled PSUM corners (all boundary fill work)
                if t + 2 < n_steps:
                    issue_warm(tiles[t + 2][0], 3, 256)
                    issue_A(t + 2, *tiles[t + 2])
                    del tiles[t]
                    if t + 3 < n_steps:
                        tiles[t + 3] = alloc_ps(t + 3)
                        issue_warm(tiles[t + 3][1], 12, 192)
                    if t + pd < n_steps:
                        nc.sync.dma_start(
                            out=ohw[:, (t + pd) % win, :], in_=oh_d[t + pd]
                        )

            issue_D(n_steps - 1)

    _split_multiwaits(nc)
    return nc


# ---------------------------------------------------------------------------
# Host-side prep + run
# ---------------------------------------------------------------------------
_CACHE = {}


def _row_perm(W):
    # [1024, N] -> [128, KC, N]: w_re[32q + f32, c, :] = W[256q + 32c + f32, :]
    N = W.shape[1]
    return np.ascontiguousarray(
        W.reshape(4, KC, 32, N).transpose(0, 2, 1, 3).reshape(128, KC, N)
    )


def _prep_inputs(x, padding_mask, emb_table, gru_kernel, gru_rec_kernel, gru_bias,
                 dense_w, dense_b, n_steps):
    x = np.asarray(x)
    padding_mask = np.asarray(padding_mask)
    emb_table = np.asarray(emb_table, dtype=np.float32)
    gru_kernel = np.asarray(gru_kernel, dtype=np.float32)
    gru_rec_kernel = np.asarray(gru_rec_kernel, dtype=np.float32)
    gru_bias = np.asarray(gru_bias, dtype=np.float32)
    dense_w = np.asarray(dense_w, dtype=np.float32)
    dense_b = np.asarray(dense_b, dtype=np.float32)

    g = emb_table @ gru_kernel  # [VOCAB, 3H]
    g = g + gru_bias[0][None, :]
    g[:, : 2 * HID] += gru_bias[1][None, : 2 * HID]

    Wz = _row_perm(gru_rec_kernel[:, :HID])
    Wr = _row_perm(gru_rec_kernel[:, HID : 2 * HID])
    Wh = _row_perm(gru_rec_kernel[:, 2 * HID :])
    wzr = np.empty((128, KC, 2048), np.float32)
    gzr = np.empty((VOCAB, 2048), np.float32)
    for j in range(Q):
        wzr[:, :, 512 * j : 512 * j + 256] = Wz[:, :, 256 * j : 256 * (j + 1)]
        wzr[:, :, 512 * j + 256 : 512 * (j + 1)] = Wr[:, :, 256 * j : 256 * (j + 1)]
        gzr[:, 512 * j : 512 * j + 256] = g[:, 256 * j : 256 * (j + 1)]
        gzr[:, 512 * j + 256 : 512 * (j + 1)] = g[:, HID + 256 * j : HID + 256 * (j + 1)]

    shared = {
        "wzr": wzr.astype(ml_dtypes.bfloat16),
        "wh": Wh.astype(ml_dtypes.bfloat16),
        "gzr": gzr.astype(ml_dtypes.bfloat16),
        "gx": np.ascontiguousarray(g[:, 2 * HID :]).astype(ml_dtypes.bfloat16),
        "b1h": gru_bias[1][None, 2 * HID :].astype(ml_dtypes.bfloat16),
        "dw": _row_perm(dense_w).astype(ml_dtypes.bfloat16),
        "db": np.ascontiguousarray(dense_b[None, :]).astype(ml_dtypes.bfloat16),
    }

    in_maps = []
    for c in range(N_CORES):
        q = c % Q
        xs = x[q * BQ : (q + 1) * BQ]
        ms = padding_mask[q * BQ : (q + 1) * BQ]
        onehot = np.zeros((n_steps, VOCAB, BQ), dtype=np.float32)
        tt = np.arange(n_steps)
        for b in range(BQ):
            onehot[tt, xs[b, :n_steps], b] = 1.0
        zbias = np.where(ms[:, :n_steps], 0.0, 30.0).astype(np.float32)  # [BQ, T]
        zbias = np.tile(zbias, (128 // BQ, 1))  # F-layout partitions
        in_maps.append(
            dict(
                shared,
                onehot=onehot.astype(ml_dtypes.bfloat16),
                zbias=np.ascontiguousarray(zbias),
            )
        )
    return in_maps


def kernel(x, padding_mask, emb_table, gru_kernel, gru_rec_kernel, gru_bias,
           dense_w, dense_b, _n_steps: int = T):
    from concourse.bass_utils import run_bass_kernel_spmd

    trace = os.environ.get("BASS_GRU_TRACE", "") == "1"
    if trace:
        _register_axon_profile_hook()

    n_steps = _n_steps
    if n_steps not in _CACHE:
        _CACHE[n_steps] = build_kernel(n_steps)
    nc = _CACHE[n_steps]

    in_maps = _prep_inputs(x, padding_mask, emb_table, gru_kernel, gru_rec_kernel,
                           gru_bias, dense_w, dense_b, n_steps)
    res = run_bass_kernel_spmd(nc, in_maps, list(range(N_CORES)), trace=trace)
    if trace:
        kernel.last_exec_time_ns = res.exec_time_ns
        print(f"HW exec time: {res.exec_time_ns} ns")

    out = np.empty((B, n_steps, VOCAB), dtype=np.float32)
    for q in range(Q):
        lg = res.results[q]["logits"]  # [n_steps, BQ, VOCAB]
        out[q * BQ : (q + 1) * BQ] = np.transpose(lg, (1, 0, 2))
    return np.ascontiguousarray(out)


kernel.last_exec_time_ns = None
